# revision 28
# baseline (speedup 1.0000x reference)
"""Causal self-attention (B=4, T=2048, C=1024, H=16) on 8 Trainium2 NeuronCores.

Core index = 2*batch + head_group: each core owns one batch element and 8 of
the 16 heads (tensor-parallel split of c_attn output dim / c_proj input dim).
Each core emits a partial projection out^T [C, T]; the host sums the two
head-group partials per batch and adds the bias terms.

fp16 datapath (fp32 PSUM accumulation everywhere, fp32 softmax denominator):
  x, W_qk, W_v, W_p are cast to fp16 on the host. fp16 weights get FWL
  (fast weight load), making per-matmul LDWEIGHTS ~4x cheaper than fp32/f32r,
  and x^T comes from a single XBAR DMA-transpose instead of 128 PE transposes.

Per-core pipeline (Tile-scheduled, phases overlap via data deps):
  A: xT = DMA-transpose(x)                       [fp16]
  B: qkT[co, tn] = W_qk^T x^T; v = x @ W_v       [fp16 matmuls, fp32 psum]
  C per head h, per 512-wide i-chunk ic:
     S^T[j, i] = k_h^T q_h   (psum groups of 2 j-tiles [128, 2, 512])
     P = exp(S^T / 8)        (one ACT op per group -> fp16)
     causal mask on diagonal groups (DVE, precomputed mask tiles)
     U'^T [65, i] (+)= [v|1]^T P^T  over j-tiles (ones column => rowsum row 64)
     yT[hd, i] = U'^T[0:64] * bcast(1/rowsum)  (ACT copies, gpsimd
                 partition_broadcast, DVE reciprocal + multiply) -> fp16
  D: out^T = W_p^T yT -> fp32 psum -> ACT copy -> DMA
"""

import numpy as np

import concourse.bass as bass
import concourse.mybir as mybir
import concourse.tile as tile
from concourse import bacc, bass_utils

B, T, C, H = 4, 2048, 1024, 16
HD = C // H          # 64 head dim
N_CORES = 8
HG = H // 2          # 8 heads per core
CL = HG * HD         # 512 local width of q/k/v
TT = T // 128        # 16 t-tiles
CB = C // 128        # 8 c-tiles
DB = CL // 128       # 4 local-hd tiles
NIC = T // 512       # i-chunks (4)

f32 = mybir.dt.float32
f16 = mybir.dt.float16

_PROG_CACHE = {}


def _emit(tc, aps):
    nc = tc.nc
    Exp = mybir.ActivationFunctionType.Exp

    x_ap = aps["x"]
    wqk_ap = aps["wqk"]
    wv_ap = aps["wv"]
    wp_ap = aps["wp"]
    bqk_ap = aps["bqk"]
    masks_ap = aps["masks"]
    outT_ap = aps["outT"]

    from contextlib import ExitStack

    with ExitStack() as outer:
        const = outer.enter_context(tc.tile_pool(name="const", bufs=1))
        p_xT = outer.enter_context(tc.tile_pool(name="xT", bufs=1))
        p_qkT = outer.enter_context(tc.tile_pool(name="qkT", bufs=1))
        p_v = outer.enter_context(tc.tile_pool(name="vv", bufs=1))
        p_yT = outer.enter_context(tc.tile_pool(name="yT", bufs=1))
        p_w = outer.enter_context(tc.tile_pool(name="wsb", bufs=1))

        # x^T via XBAR DMA-transpose first (critical path), 4 t-chunks
        xT = p_xT.tile([128, CB, T], f16)
        for tn in range(NIC):
            nc.sync.dma_start_transpose(
                xT[:, :, tn * 512 : (tn + 1) * 512],
                x_ap[tn * 512 : (tn + 1) * 512, :],
            )
        # weights on other queues; wqk split per-co so qk can start early
        wqk_sb = p_w.tile([128, CB, CB * 128], f16)  # [c-part, cb, co*128+q]
        wqk_r = wqk_ap.rearrange("(cb p) n -> p cb n", p=128)
        for co in range(CB):
            nc.gpsimd.dma_start(
                wqk_sb[:, :, co * 128 : (co + 1) * 128],
                wqk_r[:, :, co * 128 : (co + 1) * 128],
            )
        wv_sb = p_w.tile([128, CB, CL], f16)
        nc.gpsimd.dma_start(wv_sb[:], wv_ap.rearrange("(cb p) n -> p cb n", p=128))
        wp_sb = p_w.tile([128, DB, C], f16)
        nc.gpsimd.dma_start(wp_sb[:], wp_ap.rearrange("(db p) c -> p db c", p=128))
        masks = const.tile([128, 4, 512], f16)   # 1 where j > i (to be masked)
        nc.gpsimd.dma_start(masks[:], masks_ap)
        negI = const.tile([128, 128], f16)
        nc.gpsimd.dma_start(negI[:], aps["negI"])
        bqk = const.tile([128, CB], f32)
        nc.gpsimd.dma_start(bqk[:], bqk_ap.rearrange("co p -> p co"))

        # per-(co, tn) qkT tiles, per-jt v' tiles, per-tn yT tiles
        qkT = {}
        for co in range(CB):
            for tn in range(NIC):
                qkT[(co, tn)] = p_qkT.tile(
                    [128, 512], f16, tag=f"qkT_{co}_{tn}", name=f"qkT_{co}_{tn}"
                )
        vv = {}
        for jt in range(TT):
            vv[jt] = p_v.tile([128, HG, HD + 1], f16, tag=f"vv_{jt}", name=f"vv_{jt}")
            nc.vector.memset(vv[jt][:, :, HD : HD + 1], 1.0)
        yTn = {}
        for tn in range(NIC):
            yTn[tn] = p_yT.tile([128, DB, 512], f16, tag=f"yT_{tn}", name=f"yT_{tn}")

        with ExitStack() as s_all:
            ps_mm = s_all.enter_context(tc.tile_pool(name="ps_mm", bufs=2, space="PSUM"))

            # ---- B: qkv projections, tn-major so attention can start early ---
            for tn in range(NIC):
                for co in range(CB):
                    ps = ps_mm.tile([128, 512], f32, tag="mm")
                    for cb in range(CB):
                        nc.tensor.matmul(
                            ps[:],
                            wqk_sb[:, cb, co * 128 : (co + 1) * 128],
                            xT[:, cb, tn * 512 : (tn + 1) * 512],
                            start=(cb == 0),
                            stop=(cb == CB - 1),
                        )
                    nc.vector.tensor_scalar_add(qkT[(co, tn)][:], ps[:], bqk[:, co : co + 1])
                for u in range(4):
                    tt = tn * 4 + u
                    ps = ps_mm.tile([128, CL], f32, tag="mm")
                    for cb in range(CB):
                        nc.tensor.matmul(
                            ps[:],
                            xT[:, cb, tt * 128 : (tt + 1) * 128],
                            wv_sb[:, cb, :],
                            start=(cb == 0),
                            stop=(cb == CB - 1),
                        )
                    nc.scalar.activation(
                        vv[tt][:, :, 0:HD],
                        ps.rearrange("p (h d) -> p h d", d=HD),
                        mybir.ActivationFunctionType.Copy,
                    )

            # ---- C: attention + interleaved projection -----------------------
            p_p = s_all.enter_context(tc.tile_pool(name="pp", bufs=12))
            p_usb = s_all.enter_context(tc.tile_pool(name="usb", bufs=3))
            p_rb = s_all.enter_context(tc.tile_pool(name="rb", bufs=3))
            p_ost = s_all.enter_context(tc.tile_pool(name="ost", bufs=4))
            ps_sc = s_all.enter_context(tc.tile_pool(name="ps_sc", bufs=2, space="PSUM"))
            ps_u = s_all.enter_context(tc.tile_pool(name="ps_u", bufs=2, space="PSUM"))

            def normalize(h, ic, up):
                """yT[h, ic] = U'[0:64] / rowsum."""
                poff = 64 * (h % 2)
                usb = p_usb.tile([HD, 512], f32, tag="usb", name="usb")
                nc.vector.tensor_copy(usb[:], up[0:HD, :])
                rs = p_rb.tile([1, 512], f32, tag="rs", name="rs")
                nc.vector.tensor_copy(rs[:], up[HD : HD + 1, :])
                rr = p_rb.tile([1, 512], f32, tag="rr", name="rr")
                nc.vector.reciprocal_approx_fast(rr[:], rs[:])
                rb = p_rb.tile([HD, 512], f32, tag="rb", name="rb")
                nc.gpsimd.partition_broadcast(rb[:], rr[0:1, :], channels=HD)
                nc.vector.tensor_mul(
                    yTn[ic][poff : poff + HD, h // 2, :], usb[:], rb[:]
                )

            for icp in range(NIC // 2):
                ics = [2 * icp, 2 * icp + 1]
                for h in range(HG):
                    poff = 64 * (h % 2)
                    co_q = h // 2
                    co_k = 4 + h // 2
                    ups = {
                        ic: ps_u.tile([HD + 1, 512], f32, tag="u", name=f"u_{ic}")
                        for ic in ics
                    }
                    for jt in range(4 * (ics[-1] + 1)):
                        valid = [ic for ic in ics if 4 * (ic + 1) > jt]
                        nv = len(valid)
                        m = jt % 4
                        psg = ps_sc.tile([128, 2, 512], f32, tag="sc")
                        for ix, ic in enumerate(valid):
                            diag = ic == jt // 4
                            lo = 128 * m if diag else 0  # skip fully-masked cols
                            nc.tensor.matmul(
                                psg[:, ix, lo:512],
                                qkT[(co_k, jt // 4)][
                                    poff : poff + 64, m * 128 : (m + 1) * 128
                                ],
                                qkT[(co_q, ic)][poff : poff + 64, lo:512],
                                start=True,
                                stop=not diag,
                            )
                            if diag:  # -60000 above the diagonal -> exp == 0
                                nc.tensor.matmul(
                                    psg[:, ix, lo : lo + 128],
                                    negI[:],
                                    masks[:, m, lo : lo + 128],
                                    start=False,
                                    stop=True,
                                )
                        pt = p_p.tile([128, 2, 512], f16, tag="p")
                        nc.scalar.activation(
                            pt[:, 0:nv, :], psg[:, 0:nv, :], Exp, scale=1.0 / np.sqrt(HD)
                        )
                        for ix, ic in enumerate(valid):
                            diag = ic == jt // 4
                            lo = 128 * m if diag else 0
                            nc.tensor.matmul(
                                ups[ic][:, lo:512],
                                vv[jt][:, h, :],
                                pt[:, ix, lo:512],
                                start=(jt == 0),
                                stop=(jt == 4 * ic + 3),
                            )
                            if jt == 4 * ic + 3:
                                normalize(h, ic, ups[ic])

                # projection for the two finished i-chunks (overlaps attention)
                for tn in ics:
                    for co in range(CB):
                        ps = ps_mm.tile([128, 512], f32, tag="mm")
                        for db in range(DB):
                            nc.tensor.matmul(
                                ps[:],
                                wp_sb[:, db, co * 128 : (co + 1) * 128],
                                yTn[tn][:, db, :],
                                start=(db == 0),
                                stop=(db == DB - 1),
                            )
                        ot = p_ost.tile([128, 512], f32, tag="ot")
                        nc.vector.tensor_copy(ot[:], ps[:])
                        nc.sync.dma_start(
                            outT_ap[co * 128 : (co + 1) * 128, tn * 512 : (tn + 1) * 512],
                            ot[:],
                        )


def _build_program():
    nc = bacc.Bacc("TRN2", target_bir_lowering=False, debug=False, num_devices=N_CORES)
    aps = {
        "x": nc.dram_tensor("x", [T, C], f16, kind="ExternalInput").ap(),
        "wqk": nc.dram_tensor("wqk", [C, CB * 128], f16, kind="ExternalInput").ap(),
        "wv": nc.dram_tensor("wv", [C, CL], f16, kind="ExternalInput").ap(),
        "wp": nc.dram_tensor("wp", [CL, C], f16, kind="ExternalInput").ap(),
        "bqk": nc.dram_tensor("bqk", [CB, 128], f32, kind="ExternalInput").ap(),
        "masks": nc.dram_tensor("masks", [128, 4, 512], f16, kind="ExternalInput").ap(),
        "negI": nc.dram_tensor("negI", [128, 128], f16, kind="ExternalInput").ap(),
        "outT": nc.dram_tensor("outT", [C, T], f32, kind="ExternalOutput").ap(),
    }
    with tile.TileContext(nc) as tc:
        _emit(tc, aps)
    nc.compile()
    return nc


def get_program():
    if "nc" not in _PROG_CACHE:
        _PROG_CACHE["nc"] = _build_program()
    return _PROG_CACHE["nc"]


def _host_consts():
    j = np.arange(128)[:, None]
    i = np.arange(512)[None, :]
    masks = np.zeros((128, 4, 512), np.float16)
    for m in range(4):
        masks[:, m, :] = (j > i - 128 * m).astype(np.float16)  # 1 => mask out
    negI = (-60000.0 * np.eye(128)).astype(np.float16)
    return masks, negI


def make_in_maps(x, W_attn, b_attn, W_proj):
    """Build the 8 per-core input maps. Core index = 2*batch + head_group."""
    masks, negI = _host_consts()
    in_maps = []
    for core in range(N_CORES):
        b = core // 2
        g = core % 2
        wq = W_attn[:, g * CL : (g + 1) * CL]
        wk = W_attn[:, C + g * CL : C + (g + 1) * CL]
        wqk = np.concatenate([wq, wk], axis=1)  # [C, 1024], cols = co*128+q
        wv = W_attn[:, 2 * C + g * CL : 2 * C + (g + 1) * CL]
        bqk = np.concatenate(
            [b_attn[g * CL : (g + 1) * CL], b_attn[C + g * CL : C + (g + 1) * CL]]
        ).reshape(CB, 128)
        in_maps.append(
            {
                "x": np.ascontiguousarray(x[b]).astype(np.float16),
                "wqk": np.ascontiguousarray(wqk).astype(np.float16),
                "wv": np.ascontiguousarray(wv).astype(np.float16),
                "wp": np.ascontiguousarray(W_proj[g * CL : (g + 1) * CL, :]).astype(
                    np.float16
                ),
                "bqk": np.ascontiguousarray(bqk).astype(np.float32),
                "masks": masks,
                "negI": negI,
            }
        )
    return in_maps


def run(x, W_attn, b_attn, W_proj, b_proj, trace=False):
    nc = get_program()
    in_maps = make_in_maps(x, W_attn, b_attn, W_proj)
    res = bass_utils.run_bass_kernel_spmd(
        nc, in_maps, core_ids=list(range(N_CORES)), trace=trace
    )
    # combine: out[b] = sum_g outT_{2b+g}^T + (bv_g @ Wp_g summed) + b_proj
    corr = b_proj.astype(np.float64).copy()
    for g in range(2):
        bv_g = b_attn[2 * C + g * CL : 2 * C + (g + 1) * CL]
        corr += bv_g.astype(np.float64) @ W_proj[g * CL : (g + 1) * CL, :].astype(
            np.float64
        )
    out = np.empty((B, T, C), np.float32)
    for b in range(B):
        acc = (
            res.results[2 * b]["outT"].T.astype(np.float64)
            + res.results[2 * b + 1]["outT"].T.astype(np.float64)
            + corr
        )
        out[b] = acc.astype(np.float32)
    return out, res


def kernel(x, W_attn, b_attn, W_proj, b_proj):
    x = np.asarray(x, np.float32)
    W_attn = np.asarray(W_attn, np.float32)
    b_attn = np.asarray(b_attn, np.float32)
    W_proj = np.asarray(W_proj, np.float32)
    b_proj = np.asarray(b_proj, np.float32)
    out, _ = run(x, W_attn, b_attn, W_proj, b_proj)
    return out


# revision 29
# speedup vs baseline: 1.1018x; 1.1018x over previous
"""Causal self-attention (B=4, T=2048, C=1024, H=16) on 8 Trainium2 NeuronCores.

Core index = 2*batch + head_group: each core owns one batch element and 8 of
the 16 heads (tensor-parallel split of c_attn output dim / c_proj input dim).
Each core emits a partial projection out^T [C, T]; the host sums the two
head-group partials per batch and adds the bias terms.

fp16 datapath (fp32 PSUM accumulation everywhere, fp32 softmax denominator):
  x, W_qk, W_v, W_p are cast to fp16 on the host. fp16 weights get FWL
  (fast weight load), making per-matmul LDWEIGHTS ~4x cheaper than fp32/f32r,
  and x^T comes from a single XBAR DMA-transpose instead of 128 PE transposes.

Per-core pipeline (Tile-scheduled, phases overlap via data deps):
  A: xT = DMA-transpose(x)                       [fp16]
  B: qkT[co, tn] = W_qk^T x^T; v = x @ W_v       [fp16 matmuls, fp32 psum]
  C per head h, per 512-wide i-chunk ic:
     S^T[j, i] = k_h^T q_h   (psum groups of 2 j-tiles [128, 2, 512])
     P = exp(S^T / 8)        (one ACT op per group -> fp16)
     causal mask on diagonal groups (DVE, precomputed mask tiles)
     U'^T [65, i] (+)= [v|1]^T P^T  over j-tiles (ones column => rowsum row 64)
     yT[hd, i] = U'^T[0:64] * bcast(1/rowsum)  (ACT copies, gpsimd
                 partition_broadcast, DVE reciprocal + multiply) -> fp16
  D: out^T = W_p^T yT -> fp32 psum -> ACT copy -> DMA
"""

import numpy as np

import concourse.bass as bass
import concourse.mybir as mybir
import concourse.tile as tile
from concourse import bacc, bass_utils

B, T, C, H = 4, 2048, 1024, 16
HD = C // H          # 64 head dim
N_CORES = 8
HG = H // 2          # 8 heads per core
CL = HG * HD         # 512 local width of q/k/v
TT = T // 128        # 16 t-tiles
CB = C // 128        # 8 c-tiles
DB = CL // 128       # 4 local-hd tiles
NIC = T // 512       # i-chunks (4)

f32 = mybir.dt.float32
f16 = mybir.dt.float16

_PROG_CACHE = {}


def _emit(tc, aps):
    nc = tc.nc
    Exp = mybir.ActivationFunctionType.Exp

    x_ap = aps["x"]
    wqk_ap = aps["wqk"]
    wv_ap = aps["wv"]
    wp_ap = aps["wp"]
    bqk_ap = aps["bqk"]
    masks_ap = aps["masks"]
    outT_ap = aps["outT"]

    from contextlib import ExitStack

    with ExitStack() as outer:
        const = outer.enter_context(tc.tile_pool(name="const", bufs=1))
        p_xT = outer.enter_context(tc.tile_pool(name="xT", bufs=1))
        p_qkT = outer.enter_context(tc.tile_pool(name="qkT", bufs=1))
        p_v = outer.enter_context(tc.tile_pool(name="vv", bufs=1))
        p_yT = outer.enter_context(tc.tile_pool(name="yT", bufs=1))
        p_w = outer.enter_context(tc.tile_pool(name="wsb", bufs=1))

        # critical-path DMAs on sync/HWDGE: wqk then x chunks
        wqk_sb = p_w.tile([128, CB, CB * 128], f16)  # [c-part, cb, co*128+q]
        nc.sync.dma_start(wqk_sb[:], wqk_ap.rearrange("(cb p) n -> p cb n", p=128))
        xT = p_xT.tile([128, CB, T], f16)
        for tn in range(NIC):
            nc.sync.dma_start_transpose(
                xT[:, :, tn * 512 : (tn + 1) * 512],
                x_ap[tn * 512 : (tn + 1) * 512, :],
            )
        wv_sb = p_w.tile([128, CB, CL], f16)
        nc.sync.dma_start(wv_sb[:], wv_ap.rearrange("(cb p) n -> p cb n", p=128))
        wp_sb = p_w.tile([128, DB, C], f16)
        nc.sync.dma_start(wp_sb[:], wp_ap.rearrange("(db p) c -> p db c", p=128))
        masks = const.tile([128, 4, 512], f16)   # 1 where j > i (to be masked)
        nc.gpsimd.dma_start(masks[:], masks_ap)
        negI = const.tile([128, 128], f16)
        nc.gpsimd.dma_start(negI[:], aps["negI"])
        bqk = const.tile([128, CB], f32)
        nc.gpsimd.dma_start(bqk[:], bqk_ap.rearrange("co p -> p co"))

        # per-(co, tn) qkT tiles, per-jt v' tiles, per-tn yT tiles
        qkT = {}
        for co in range(CB):
            for tn in range(NIC):
                qkT[(co, tn)] = p_qkT.tile(
                    [128, 512], f16, tag=f"qkT_{co}_{tn}", name=f"qkT_{co}_{tn}"
                )
        vv = {}
        for jt in range(TT):
            vv[jt] = p_v.tile([128, HG, HD + 1], f16, tag=f"vv_{jt}", name=f"vv_{jt}")
            nc.vector.memset(vv[jt][:, :, HD : HD + 1], 1.0)
        yTn = {}
        for tn in range(NIC):
            yTn[tn] = p_yT.tile([128, DB, 512], f16, tag=f"yT_{tn}", name=f"yT_{tn}")

        with ExitStack() as s_all:
            ps_ab = ExitStack()
            ps_mm = ps_ab.enter_context(tc.tile_pool(name="ps_mm", bufs=4, space="PSUM"))

            # ---- B: qkv projections, tn-major so attention can start early ---
            for tn in range(NIC):
                for co in range(CB):
                    ps = ps_mm.tile([128, 512], f32, tag="mm")
                    for cb in range(CB):
                        nc.tensor.matmul(
                            ps[:],
                            wqk_sb[:, cb, co * 128 : (co + 1) * 128],
                            xT[:, cb, tn * 512 : (tn + 1) * 512],
                            start=(cb == 0),
                            stop=(cb == CB - 1),
                        )
                    nc.vector.tensor_scalar_add(qkT[(co, tn)][:], ps[:], bqk[:, co : co + 1])
                for u in range(4):
                    tt = tn * 4 + u
                    ps = ps_mm.tile([128, CL], f32, tag="mm")
                    for cb in range(CB):
                        nc.tensor.matmul(
                            ps[:],
                            xT[:, cb, tt * 128 : (tt + 1) * 128],
                            wv_sb[:, cb, :],
                            start=(cb == 0),
                            stop=(cb == CB - 1),
                        )
                    nc.scalar.activation(
                        vv[tt][:, :, 0:HD],
                        ps.rearrange("p (h d) -> p h d", d=HD),
                        mybir.ActivationFunctionType.Copy,
                    )

            ps_ab.close()  # free A/B psum banks

            # ---- C: attention + interleaved projection -----------------------
            p_p = s_all.enter_context(tc.tile_pool(name="pp", bufs=12))
            p_usb = s_all.enter_context(tc.tile_pool(name="usb", bufs=3))
            p_rb = s_all.enter_context(tc.tile_pool(name="rb", bufs=3))
            p_ost = s_all.enter_context(tc.tile_pool(name="ost", bufs=4))
            ps_sc = s_all.enter_context(tc.tile_pool(name="ps_sc", bufs=2, space="PSUM"))
            ps_u = s_all.enter_context(tc.tile_pool(name="ps_u", bufs=2, space="PSUM"))

            def normalize(h, ic, up):
                """yT[h, ic] = U'[0:64] / rowsum."""
                poff = 64 * (h % 2)
                usb = p_usb.tile([HD, 512], f32, tag="usb", name="usb")
                nc.vector.tensor_copy(usb[:], up[0:HD, :])
                rs = p_rb.tile([1, 512], f32, tag="rs", name="rs")
                nc.vector.tensor_copy(rs[:], up[HD : HD + 1, :])
                rr = p_rb.tile([1, 512], f32, tag="rr", name="rr")
                nc.vector.reciprocal_approx_fast(rr[:], rs[:])
                rb = p_rb.tile([HD, 512], f32, tag="rb", name="rb")
                nc.gpsimd.partition_broadcast(rb[:], rr[0:1, :], channels=HD)
                nc.vector.tensor_mul(
                    yTn[ic][poff : poff + HD, h // 2, :], usb[:], rb[:]
                )

            for icp in range(NIC // 2):
                ics = [2 * icp, 2 * icp + 1]
                for h in range(HG):
                    poff = 64 * (h % 2)
                    co_q = h // 2
                    co_k = 4 + h // 2
                    ups = {
                        ic: ps_u.tile([HD + 1, 512], f32, tag="u", name=f"u_{ic}")
                        for ic in ics
                    }
                    for jt in range(4 * (ics[-1] + 1)):
                        valid = [ic for ic in ics if 4 * (ic + 1) > jt]
                        nv = len(valid)
                        m = jt % 4
                        psg = ps_sc.tile([128, 2, 512], f32, tag="sc")
                        for ix, ic in enumerate(valid):
                            diag = ic == jt // 4
                            lo = 128 * m if diag else 0  # skip fully-masked cols
                            nc.tensor.matmul(
                                psg[:, ix, lo:512],
                                qkT[(co_k, jt // 4)][
                                    poff : poff + 64, m * 128 : (m + 1) * 128
                                ],
                                qkT[(co_q, ic)][poff : poff + 64, lo:512],
                                start=True,
                                stop=not diag,
                            )
                            if diag:  # -60000 above the diagonal -> exp == 0
                                nc.tensor.matmul(
                                    psg[:, ix, lo : lo + 128],
                                    negI[:],
                                    masks[:, m, lo : lo + 128],
                                    start=False,
                                    stop=True,
                                )
                        pt = p_p.tile([128, 2, 512], f16, tag="p")
                        nc.scalar.activation(
                            pt[:, 0:nv, :], psg[:, 0:nv, :], Exp, scale=1.0 / np.sqrt(HD)
                        )
                        for ix, ic in enumerate(valid):
                            diag = ic == jt // 4
                            lo = 128 * m if diag else 0
                            nc.tensor.matmul(
                                ups[ic][:, lo:512],
                                vv[jt][:, h, :],
                                pt[:, ix, lo:512],
                                start=(jt == 0),
                                stop=(jt == 4 * ic + 3),
                            )
                            if jt == 4 * ic + 3:
                                normalize(h, ic, ups[ic])

                # projection for the two finished i-chunks (overlaps attention)
                for tn in ics:
                    for co in range(CB):
                        ps = ps_sc.tile([128, 512], f32, tag="sc1")
                        for db in range(DB):
                            nc.tensor.matmul(
                                ps[:],
                                wp_sb[:, db, co * 128 : (co + 1) * 128],
                                yTn[tn][:, db, :],
                                start=(db == 0),
                                stop=(db == DB - 1),
                            )
                        ot = p_ost.tile([128, 512], f32, tag="ot")
                        nc.vector.tensor_copy(ot[:], ps[:])
                        nc.sync.dma_start(
                            outT_ap[co * 128 : (co + 1) * 128, tn * 512 : (tn + 1) * 512],
                            ot[:],
                        )


def _build_program():
    nc = bacc.Bacc("TRN2", target_bir_lowering=False, debug=False, num_devices=N_CORES)
    aps = {
        "x": nc.dram_tensor("x", [T, C], f16, kind="ExternalInput").ap(),
        "wqk": nc.dram_tensor("wqk", [C, CB * 128], f16, kind="ExternalInput").ap(),
        "wv": nc.dram_tensor("wv", [C, CL], f16, kind="ExternalInput").ap(),
        "wp": nc.dram_tensor("wp", [CL, C], f16, kind="ExternalInput").ap(),
        "bqk": nc.dram_tensor("bqk", [CB, 128], f32, kind="ExternalInput").ap(),
        "masks": nc.dram_tensor("masks", [128, 4, 512], f16, kind="ExternalInput").ap(),
        "negI": nc.dram_tensor("negI", [128, 128], f16, kind="ExternalInput").ap(),
        "outT": nc.dram_tensor("outT", [C, T], f32, kind="ExternalOutput").ap(),
    }
    with tile.TileContext(nc) as tc:
        _emit(tc, aps)
    nc.compile()
    return nc


def get_program():
    if "nc" not in _PROG_CACHE:
        _PROG_CACHE["nc"] = _build_program()
    return _PROG_CACHE["nc"]


def _host_consts():
    j = np.arange(128)[:, None]
    i = np.arange(512)[None, :]
    masks = np.zeros((128, 4, 512), np.float16)
    for m in range(4):
        masks[:, m, :] = (j > i - 128 * m).astype(np.float16)  # 1 => mask out
    negI = (-60000.0 * np.eye(128)).astype(np.float16)
    return masks, negI


def make_in_maps(x, W_attn, b_attn, W_proj):
    """Build the 8 per-core input maps. Core index = 2*batch + head_group."""
    masks, negI = _host_consts()
    in_maps = []
    for core in range(N_CORES):
        b = core // 2
        g = core % 2
        wq = W_attn[:, g * CL : (g + 1) * CL]
        wk = W_attn[:, C + g * CL : C + (g + 1) * CL]
        wqk = np.concatenate([wq, wk], axis=1)  # [C, 1024], cols = co*128+q
        wv = W_attn[:, 2 * C + g * CL : 2 * C + (g + 1) * CL]
        bqk = np.concatenate(
            [b_attn[g * CL : (g + 1) * CL], b_attn[C + g * CL : C + (g + 1) * CL]]
        ).reshape(CB, 128)
        in_maps.append(
            {
                "x": np.ascontiguousarray(x[b]).astype(np.float16),
                "wqk": np.ascontiguousarray(wqk).astype(np.float16),
                "wv": np.ascontiguousarray(wv).astype(np.float16),
                "wp": np.ascontiguousarray(W_proj[g * CL : (g + 1) * CL, :]).astype(
                    np.float16
                ),
                "bqk": np.ascontiguousarray(bqk).astype(np.float32),
                "masks": masks,
                "negI": negI,
            }
        )
    return in_maps


def run(x, W_attn, b_attn, W_proj, b_proj, trace=False):
    nc = get_program()
    in_maps = make_in_maps(x, W_attn, b_attn, W_proj)
    res = bass_utils.run_bass_kernel_spmd(
        nc, in_maps, core_ids=list(range(N_CORES)), trace=trace
    )
    # combine: out[b] = sum_g outT_{2b+g}^T + (bv_g @ Wp_g summed) + b_proj
    corr = b_proj.astype(np.float64).copy()
    for g in range(2):
        bv_g = b_attn[2 * C + g * CL : 2 * C + (g + 1) * CL]
        corr += bv_g.astype(np.float64) @ W_proj[g * CL : (g + 1) * CL, :].astype(
            np.float64
        )
    out = np.empty((B, T, C), np.float32)
    for b in range(B):
        acc = (
            res.results[2 * b]["outT"].T.astype(np.float64)
            + res.results[2 * b + 1]["outT"].T.astype(np.float64)
            + corr
        )
        out[b] = acc.astype(np.float32)
    return out, res


def kernel(x, W_attn, b_attn, W_proj, b_proj):
    x = np.asarray(x, np.float32)
    W_attn = np.asarray(W_attn, np.float32)
    b_attn = np.asarray(b_attn, np.float32)
    W_proj = np.asarray(W_proj, np.float32)
    b_proj = np.asarray(b_proj, np.float32)
    out, _ = run(x, W_attn, b_attn, W_proj, b_proj)
    return out


# revision 30
# speedup vs baseline: 1.1778x; 1.0689x over previous
"""Causal self-attention (B=4, T=2048, C=1024, H=16) on 8 Trainium2 NeuronCores.

Core index = 2*batch + head_group: each core owns one batch element and 8 of
the 16 heads (tensor-parallel split of c_attn output dim / c_proj input dim).
Each core emits a partial projection out^T [C, T]; the host sums the two
head-group partials per batch and adds the bias terms.

fp16 datapath (fp32 PSUM accumulation everywhere, fp32 softmax denominator):
  x, W_qk, W_v, W_p are cast to fp16 on the host. fp16 weights get FWL
  (fast weight load), making per-matmul LDWEIGHTS ~4x cheaper than fp32/f32r,
  and x^T comes from a single XBAR DMA-transpose instead of 128 PE transposes.

Per-core pipeline (Tile-scheduled, phases overlap via data deps):
  A: xT = DMA-transpose(x)                       [fp16]
  B: qkT[co, tn] = W_qk^T x^T; v = x @ W_v       [fp16 matmuls, fp32 psum]
  C per head h, per 512-wide i-chunk ic:
     S^T[j, i] = k_h^T q_h   (psum groups of 2 j-tiles [128, 2, 512])
     P = exp(S^T / 8)        (one ACT op per group -> fp16)
     causal mask on diagonal groups (DVE, precomputed mask tiles)
     U'^T [65, i] (+)= [v|1]^T P^T  over j-tiles (ones column => rowsum row 64)
     yT[hd, i] = U'^T[0:64] * bcast(1/rowsum)  (ACT copies, gpsimd
                 partition_broadcast, DVE reciprocal + multiply) -> fp16
  D: out^T = W_p^T yT -> fp32 psum -> ACT copy -> DMA
"""

import numpy as np

import concourse.bass as bass
import concourse.mybir as mybir
import concourse.tile as tile
from concourse import bacc, bass_utils

B, T, C, H = 4, 2048, 1024, 16
HD = C // H          # 64 head dim
N_CORES = 8
HG = H // 2          # 8 heads per core
CL = HG * HD         # 512 local width of q/k/v
TT = T // 128        # 16 t-tiles
CB = C // 128        # 8 c-tiles
DB = CL // 128       # 4 local-hd tiles
NIC = T // 512       # i-chunks (4)

f32 = mybir.dt.float32
f16 = mybir.dt.float16

_PROG_CACHE = {}


def _emit(tc, aps):
    nc = tc.nc
    Exp = mybir.ActivationFunctionType.Exp

    x_ap = aps["x"]
    wqk_ap = aps["wqk"]
    wv_ap = aps["wv"]
    wp_ap = aps["wp"]
    bqk_ap = aps["bqk"]
    masks_ap = aps["masks"]
    outT_ap = aps["outT"]

    from contextlib import ExitStack

    with ExitStack() as outer:
        const = outer.enter_context(tc.tile_pool(name="const", bufs=1))
        p_xT = outer.enter_context(tc.tile_pool(name="xT", bufs=1))
        p_qkT = outer.enter_context(tc.tile_pool(name="qkT", bufs=1))
        p_v = outer.enter_context(tc.tile_pool(name="vv", bufs=1))
        p_yT = outer.enter_context(tc.tile_pool(name="yT", bufs=1))
        p_w = outer.enter_context(tc.tile_pool(name="wsb", bufs=1))

        # critical-path DMAs on sync/HWDGE: wqk then x chunks
        wqk_sb = p_w.tile([128, CB, CB * 128], f16)  # [c-part, cb, co*128+q]
        nc.sync.dma_start(wqk_sb[:], wqk_ap.rearrange("(cb p) n -> p cb n", p=128))
        xT = p_xT.tile([128, CB, T], f16)
        for tn in range(NIC):
            nc.sync.dma_start_transpose(
                xT[:, :, tn * 512 : (tn + 1) * 512],
                x_ap[tn * 512 : (tn + 1) * 512, :],
            )
        wv_sb = p_w.tile([128, CB, CL], f16)
        nc.sync.dma_start(wv_sb[:], wv_ap.rearrange("(cb p) n -> p cb n", p=128))
        wp_sb = p_w.tile([128, DB, C], f16)
        nc.sync.dma_start(wp_sb[:], wp_ap.rearrange("(db p) c -> p db c", p=128))
        masks = const.tile([128, 4, 512], f16)   # 1 where j > i (to be masked)
        nc.gpsimd.dma_start(masks[:], masks_ap)
        negI = const.tile([128, 128], f16)
        nc.gpsimd.dma_start(negI[:], aps["negI"])
        bqk = const.tile([128, CB], f32)
        nc.gpsimd.dma_start(bqk[:], bqk_ap.rearrange("co p -> p co"))

        # per-(co, tn) qkT tiles, per-jt v' tiles, per-tn yT tiles
        qkT = {}
        for co in range(CB):
            for tn in range(NIC):
                qkT[(co, tn)] = p_qkT.tile(
                    [128, 512], f16, tag=f"qkT_{co}_{tn}", name=f"qkT_{co}_{tn}"
                )
        vv = {}
        for jt in range(TT):
            vv[jt] = p_v.tile([128, HG, HD + 1], f16, tag=f"vv_{jt}", name=f"vv_{jt}")
            nc.vector.memset(vv[jt][:, :, HD : HD + 1], 1.0)
        yTn = {}
        for tn in range(NIC):
            yTn[tn] = p_yT.tile([128, DB, 512], f16, tag=f"yT_{tn}", name=f"yT_{tn}")

        with ExitStack() as s_all:
            ps_ab = ExitStack()
            ps_mm = ps_ab.enter_context(tc.tile_pool(name="ps_mm", bufs=4, space="PSUM"))

            # ---- B: qkv projections, tn-major so attention can start early ---
            for tn in range(NIC):
                for co in range(CB):
                    ps = ps_mm.tile([128, 512], f32, tag="mm")
                    for cb in range(CB):
                        nc.tensor.matmul(
                            ps[:],
                            wqk_sb[:, cb, co * 128 : (co + 1) * 128],
                            xT[:, cb, tn * 512 : (tn + 1) * 512],
                            start=(cb == 0),
                            stop=(cb == CB - 1),
                        )
                    nc.vector.tensor_scalar_add(qkT[(co, tn)][:], ps[:], bqk[:, co : co + 1])
                for u in range(4):
                    tt = tn * 4 + u
                    ps = ps_mm.tile([128, CL], f32, tag="mm")
                    for cb in range(CB):
                        nc.tensor.matmul(
                            ps[:],
                            xT[:, cb, tt * 128 : (tt + 1) * 128],
                            wv_sb[:, cb, :],
                            start=(cb == 0),
                            stop=(cb == CB - 1),
                        )
                    nc.scalar.activation(
                        vv[tt][:, :, 0:HD],
                        ps.rearrange("p (h d) -> p h d", d=HD),
                        mybir.ActivationFunctionType.Copy,
                    )

            ps_ab.close()  # free A/B psum banks

            # ---- C: attention + interleaved projection -----------------------
            p_p = s_all.enter_context(tc.tile_pool(name="pp", bufs=12))
            p_usb = s_all.enter_context(tc.tile_pool(name="usb", bufs=3))
            p_rb = s_all.enter_context(tc.tile_pool(name="rb", bufs=3))
            p_ost = s_all.enter_context(tc.tile_pool(name="ost", bufs=4))
            ps_sc = s_all.enter_context(tc.tile_pool(name="ps_sc", bufs=3, space="PSUM"))
            ps_u = s_all.enter_context(tc.tile_pool(name="ps_u", bufs=2, space="PSUM"))

            def normalize(h, ic, up):
                """yT[h, ic] = U'[0:64] / rowsum."""
                poff = 64 * (h % 2)
                usb = p_usb.tile([HD, 512], f32, tag="usb", name="usb")
                nc.vector.tensor_copy(usb[:], up[0:HD, :])
                rs = p_rb.tile([1, 512], f32, tag="rs", name="rs")
                nc.vector.tensor_copy(rs[:], up[HD : HD + 1, :])
                rr = p_rb.tile([1, 512], f32, tag="rr", name="rr")
                nc.vector.reciprocal_approx_fast(rr[:], rs[:])
                rb = p_rb.tile([HD, 512], f32, tag="rb", name="rb")
                nc.gpsimd.partition_broadcast(rb[:], rr[0:1, :], channels=HD)
                nc.vector.tensor_mul(
                    yTn[ic][poff : poff + HD, h // 2, :], usb[:], rb[:]
                )

            for icp in range(NIC // 2):
                ics = [2 * icp, 2 * icp + 1]
                for h in range(HG):
                    poff = 64 * (h % 2)
                    co_q = h // 2
                    co_k = 4 + h // 2
                    ups = {
                        ic: ps_u.tile([HD + 1, 512], f32, tag="u", name=f"u_{ic}")
                        for ic in ics
                    }
                    # all (jt, ic) sub-tiles in jt-major order, packed in pairs
                    subs = [
                        (jt, ic)
                        for jt in range(4 * (ics[-1] + 1))
                        for ic in ics
                        if 4 * (ic + 1) > jt
                    ]
                    for g0 in range(0, len(subs), 2):
                        grp = subs[g0 : g0 + 2]
                        psg = ps_sc.tile([128, 2, 512], f32, tag="sc")
                        for ix, (jt, ic) in enumerate(grp):
                            m = jt % 4
                            diag = ic == jt // 4
                            lo = 128 * m if diag else 0
                            nc.tensor.matmul(
                                psg[:, ix, lo:512],
                                qkT[(co_k, jt // 4)][
                                    poff : poff + 64, m * 128 : (m + 1) * 128
                                ],
                                qkT[(co_q, ic)][poff : poff + 64, lo:512],
                                start=True,
                                stop=not diag,
                            )
                            if diag:  # -60000 above the diagonal -> exp == 0
                                nc.tensor.matmul(
                                    psg[:, ix, lo : lo + 128],
                                    negI[:],
                                    masks[:, m, lo : lo + 128],
                                    start=False,
                                    stop=True,
                                )
                        pt = p_p.tile([128, 2, 512], f16, tag="p")
                        nv = len(grp)
                        nc.scalar.activation(
                            pt[:, 0:nv, :], psg[:, 0:nv, :], Exp, scale=1.0 / np.sqrt(HD)
                        )
                        for ix, (jt, ic) in enumerate(grp):
                            m = jt % 4
                            diag = ic == jt // 4
                            lo = 128 * m if diag else 0
                            nc.tensor.matmul(
                                ups[ic][:, lo:512],
                                vv[jt][:, h, :],
                                pt[:, ix, lo:512],
                                start=(jt == 0),
                                stop=(jt == 4 * ic + 3),
                            )
                            if jt == 4 * ic + 3:
                                normalize(h, ic, ups[ic])

                # projection for the two finished i-chunks (overlaps attention)
                for co in range(CB):
                    psp = ps_sc.tile([128, 2, 512], f32, tag="sc")
                    for ix, tn in enumerate(ics):
                        for db in range(DB):
                            nc.tensor.matmul(
                                psp[:, ix, :],
                                wp_sb[:, db, co * 128 : (co + 1) * 128],
                                yTn[tn][:, db, :],
                                start=(db == 0),
                                stop=(db == DB - 1),
                            )
                    ot = p_ost.tile([128, 2, 512], f32, tag="ot")
                    nc.vector.tensor_copy(ot[:], psp[:])
                    for ix, tn in enumerate(ics):
                        nc.sync.dma_start(
                            outT_ap[co * 128 : (co + 1) * 128, tn * 512 : (tn + 1) * 512],
                            ot[:, ix, :],
                        )


def _build_program():
    nc = bacc.Bacc("TRN2", target_bir_lowering=False, debug=False, num_devices=N_CORES)
    aps = {
        "x": nc.dram_tensor("x", [T, C], f16, kind="ExternalInput").ap(),
        "wqk": nc.dram_tensor("wqk", [C, CB * 128], f16, kind="ExternalInput").ap(),
        "wv": nc.dram_tensor("wv", [C, CL], f16, kind="ExternalInput").ap(),
        "wp": nc.dram_tensor("wp", [CL, C], f16, kind="ExternalInput").ap(),
        "bqk": nc.dram_tensor("bqk", [CB, 128], f32, kind="ExternalInput").ap(),
        "masks": nc.dram_tensor("masks", [128, 4, 512], f16, kind="ExternalInput").ap(),
        "negI": nc.dram_tensor("negI", [128, 128], f16, kind="ExternalInput").ap(),
        "outT": nc.dram_tensor("outT", [C, T], f32, kind="ExternalOutput").ap(),
    }
    with tile.TileContext(nc) as tc:
        _emit(tc, aps)
    nc.compile()
    return nc


def get_program():
    if "nc" not in _PROG_CACHE:
        _PROG_CACHE["nc"] = _build_program()
    return _PROG_CACHE["nc"]


def _host_consts():
    j = np.arange(128)[:, None]
    i = np.arange(512)[None, :]
    masks = np.zeros((128, 4, 512), np.float16)
    for m in range(4):
        masks[:, m, :] = (j > i - 128 * m).astype(np.float16)  # 1 => mask out
    negI = (-60000.0 * np.eye(128)).astype(np.float16)
    return masks, negI


def make_in_maps(x, W_attn, b_attn, W_proj):
    """Build the 8 per-core input maps. Core index = 2*batch + head_group."""
    masks, negI = _host_consts()
    in_maps = []
    for core in range(N_CORES):
        b = core // 2
        g = core % 2
        wq = W_attn[:, g * CL : (g + 1) * CL]
        wk = W_attn[:, C + g * CL : C + (g + 1) * CL]
        wqk = np.concatenate([wq, wk], axis=1)  # [C, 1024], cols = co*128+q
        wv = W_attn[:, 2 * C + g * CL : 2 * C + (g + 1) * CL]
        bqk = np.concatenate(
            [b_attn[g * CL : (g + 1) * CL], b_attn[C + g * CL : C + (g + 1) * CL]]
        ).reshape(CB, 128)
        in_maps.append(
            {
                "x": np.ascontiguousarray(x[b]).astype(np.float16),
                "wqk": np.ascontiguousarray(wqk).astype(np.float16),
                "wv": np.ascontiguousarray(wv).astype(np.float16),
                "wp": np.ascontiguousarray(W_proj[g * CL : (g + 1) * CL, :]).astype(
                    np.float16
                ),
                "bqk": np.ascontiguousarray(bqk).astype(np.float32),
                "masks": masks,
                "negI": negI,
            }
        )
    return in_maps


def run(x, W_attn, b_attn, W_proj, b_proj, trace=False):
    nc = get_program()
    in_maps = make_in_maps(x, W_attn, b_attn, W_proj)
    res = bass_utils.run_bass_kernel_spmd(
        nc, in_maps, core_ids=list(range(N_CORES)), trace=trace
    )
    # combine: out[b] = sum_g outT_{2b+g}^T + (bv_g @ Wp_g summed) + b_proj
    corr = b_proj.astype(np.float64).copy()
    for g in range(2):
        bv_g = b_attn[2 * C + g * CL : 2 * C + (g + 1) * CL]
        corr += bv_g.astype(np.float64) @ W_proj[g * CL : (g + 1) * CL, :].astype(
            np.float64
        )
    out = np.empty((B, T, C), np.float32)
    for b in range(B):
        acc = (
            res.results[2 * b]["outT"].T.astype(np.float64)
            + res.results[2 * b + 1]["outT"].T.astype(np.float64)
            + corr
        )
        out[b] = acc.astype(np.float32)
    return out, res


def kernel(x, W_attn, b_attn, W_proj, b_proj):
    x = np.asarray(x, np.float32)
    W_attn = np.asarray(W_attn, np.float32)
    b_attn = np.asarray(b_attn, np.float32)
    W_proj = np.asarray(W_proj, np.float32)
    b_proj = np.asarray(b_proj, np.float32)
    out, _ = run(x, W_attn, b_attn, W_proj, b_proj)
    return out
